# revision 6
# baseline (speedup 1.0000x reference)
"""Trainium2 Bass kernel for nn_PSN (gnn_message_passing), 8 NeuronCores.

Math (per reference):
    deg  = segment_sum(ones, col); deg[deg==0] = 1; dis = deg^-1/2
    repeat L times:  agg = scatter_add(col, dis[row]*dis[col]*cur[row]);
                     cur = cur - agg; update += tanh(k_i)*cur
    c = sigmoid(weighting); h = c*update + (1-c)*x; out = relu(h @ W.T + b)

Strategy: shard nodes across 8 cores (1280/core, 10 blocks of 128).
Host sorts edges by target (col), pads each 128-node block's edge list to
C chunks of 128 edges, and encodes target offsets as 0/1 one-hot matrices
(bf16).  Per layer on device:
    y = dis * cur  (bf16)  -> AllGather (all cores get full y in HBM)
    per block: dma_gather rows y[row[e]] (edges-on-partitions), then
    C accumulating matmuls  psum[node,feat] += S01[e,node].T @ msg[e,feat]
    fused drain: cur_b += psum * (-dis_b)
    update += tanh(k_i) * cur
Epilogue: h = x + c*(update-x); PE-transpose h; out^T = W @ h^T (+b, relu).
"""
import sys
import types
import numpy as np
import ml_dtypes
from contextlib import ExitStack

import concourse.bass as bass
import concourse.tile as tile
from concourse import bacc, mybir
from concourse.bass_utils import run_bass_kernel_spmd

P = 128          # partitions / block size
NC = 8           # cores
F = 128          # feature dim (must equal P for this kernel)
L = 8            # layers

FP32 = mybir.dt.float32
BF16 = mybir.dt.bfloat16
I16 = mybir.dt.int16

LAST_EXEC_TIME_NS = None


def _install_ntff_hook():
    """Optional: register the axon NTFF profile hook so BASS_TRACE=1 yields
    exec_time_ns.  Silently no-ops in environments without the axon boot."""
    try:
        if "antenv.axon_hooks" in sys.modules:
            return
        import antenv
        from trn_agent_boot.trn_boot import _ntff_profile_via_ctypes

        m = types.ModuleType("antenv.axon_hooks")
        _state = {"hook": _ntff_profile_via_ctypes("/opt/axon/libaxon_pjrt.so")}
        m.set_axon_ntff_profile_hook = lambda h: _state.__setitem__("hook", h)
        m.get_axon_ntff_profile_hook = lambda: _state["hook"]
        sys.modules["antenv.axon_hooks"] = m
        antenv.axon_hooks = m
    except Exception:
        pass


def _wrap_idx(idx: np.ndarray) -> np.ndarray:
    """int16 index layout for dma_gather: i -> [i%16, i//16], replicated to
    all 8 gpsimd core groups (128 partitions)."""
    n = idx.shape[0]
    assert n % 16 == 0
    w = idx.reshape(n // 16, 16).T.astype(np.int16)  # [16, n/16]
    return np.tile(w, (8, 1))                        # [128, n/16]


def preprocess(x, edge_index):
    """Host-side (numpy) index/layout preprocessing. Returns per-core input
    arrays and the chunk count C."""
    N = x.shape[0]
    E = edge_index.shape[1]
    NB = -(-N // (NC * P))           # blocks per core
    NPC = NB * P                     # padded nodes per core
    NTOT = NC * NPC                  # AllGather rows
    assert NTOT < 2 ** 15, "int16 gather indices"

    row = np.asarray(edge_index[0], dtype=np.int64)
    col = np.asarray(edge_index[1], dtype=np.int64)
    order = np.argsort(col, kind="stable")
    row_s, col_s = row[order], col[order]

    G = NC * NB                      # total padded blocks
    blk = col_s // P                 # block of each edge (aligned since NPC % P == 0)
    counts = np.bincount(blk, minlength=G)
    C = max(1, int(-(-counts.max() // P)))   # chunks per block (uniform)
    EPB = C * P

    # per-block padded row ids and local col offsets
    rows_pad = np.zeros((G, EPB), dtype=np.int64)
    lcol_pad = np.full((G, EPB), -1, dtype=np.int64)
    starts = np.concatenate([[0], np.cumsum(counts)])
    for g in range(G):
        n = counts[g]
        if n:
            sl = slice(starts[g], starts[g] + n)
            rows_pad[g, :n] = row_s[sl]
            lcol_pad[g, :n] = col_s[sl] - g * P

    # one-hot S01 [G, P(edge-in-chunk), C, P(node)] bf16
    lc = lcol_pad.reshape(G, C, P)                       # (g, k, p)
    onehot = (lc[..., None] == np.arange(P)).astype(ml_dtypes.bfloat16)  # (g,k,p,m)
    s01 = onehot.transpose(0, 2, 1, 3)                   # (g, p, k, m)

    x_pad = np.zeros((NTOT, F), dtype=np.float32)
    x_pad[:N] = np.asarray(x, dtype=np.float32)

    per_core = []
    for r in range(NC):
        gs = slice(r * NB, (r + 1) * NB)
        s01_r = s01[gs].transpose(1, 0, 2, 3).reshape(P, NB * C * P)  # (p, b*k*m)
        s01_r = np.ascontiguousarray(s01_r)
        gidx_r = np.concatenate(
            [_wrap_idx(rows_pad[g]) for g in range(r * NB, (r + 1) * NB)], axis=1
        )                                                  # [128, NB*C*8]
        xs_r = np.ascontiguousarray(
            x_pad[r * NPC:(r + 1) * NPC].reshape(NB, P, F).transpose(1, 0, 2)
            .reshape(P, NB * F)
        )                                                  # [128, NB*128]
        per_core.append({"s01": s01_r, "gidx": gidx_r, "x_sl": xs_r})
    return per_core, C, NB, NPC, NTOT, N


BUILD_L = L
Y_SHARED = True
SKIP_GATHER = False
SKIP_CC = False


def build_program(C, NB):
    NPC = NB * P
    NTOT = NC * NPC
    nlayers = BUILD_L
    nc = bacc.Bacc("TRN2", target_bir_lowering=False, debug=False,
                   enable_asserts=False, num_devices=NC)

    x_in = nc.dram_tensor("x_sl", [P, NB * F], FP32, kind="ExternalInput")
    s01_in = nc.dram_tensor("s01", [P, NB * C * P], BF16, kind="ExternalInput")
    gidx_in = nc.dram_tensor("gidx", [P, NB * C * 8], I16, kind="ExternalInput")
    wt_in = nc.dram_tensor("wt", [F, F], FP32, kind="ExternalInput")      # W.T
    bias_in = nc.dram_tensor("bias", [F, 1], FP32, kind="ExternalInput")
    kv_in = nc.dram_tensor("kv", [1, L], FP32, kind="ExternalInput")
    wg_in = nc.dram_tensor("wg", [1, 1], FP32, kind="ExternalInput")
    id_in = nc.dram_tensor("ident", [P, P], FP32, kind="ExternalInput")
    out_t = nc.dram_tensor("outT", [F, NPC], FP32, kind="ExternalOutput")

    with tile.TileContext(nc) as tc, ExitStack() as ctx:
        per = ctx.enter_context(tc.tile_pool(name="per", bufs=1))       # persistent
        msgp = ctx.enter_context(tc.tile_pool(name="msgp", bufs=3))
        wk = ctx.enter_context(tc.tile_pool(name="wk", bufs=2))
        ps_agg = ctx.enter_context(tc.tile_pool(name="ps_agg", bufs=2, space="PSUM"))
        ps_m = ctx.enter_context(tc.tile_pool(name="ps_m", bufs=2, space="PSUM"))
        ps_f = ctx.enter_context(tc.tile_pool(name="ps_f", bufs=2, space="PSUM"))
        dram = ctx.enter_context(tc.tile_pool(name="dram", bufs=1, space="DRAM"))

        # ---- persistent SBUF state ----
        s01_sb = per.tile([P, NB * C * P], BF16)
        gidx_sb = per.tile([P, NB * C * 8], I16)
        xs = per.tile([P, NB * F], FP32)
        cur = per.tile([P, NB * F], FP32)
        upd = per.tile([P, NB * F], FP32)
        disx = per.tile([P, NB * F], FP32)
        y_sb = per.tile([P, NB * F], BF16)
        dis = per.tile([P, NB], FP32)
        ndis = per.tile([P, NB], FP32)
        tanhk = per.tile([P, L], FP32)
        cbc = per.tile([P, 1], FP32)
        wt_sb = per.tile([F, F], FP32)
        id_sb = per.tile([P, P], FP32)
        bias_sb = per.tile([F, 1], FP32)
        ones1 = per.tile([1, P], FP32)
        ones_t = per.tile([P, P], FP32)
        onesb = per.tile([P, 1], BF16)
        hT = per.tile([F, NPC], FP32)
        outsb = per.tile([F, NPC], FP32)

        # DRAM bounce buffers for the collective
        y_in = dram.tile([NPC, F], BF16)
        y_out_t = nc.dram_tensor("y_out_sh", [NTOT, F], BF16,
                                 addr_space="Shared" if Y_SHARED else "Local")
        y_out = y_out_t.ap()

        # ---- loads ----
        nc.sync.dma_start(s01_sb[:], s01_in[:])
        nc.sync.dma_start(gidx_sb[:], gidx_in[:])
        nc.sync.dma_start(xs[:], x_in[:])
        nc.sync.dma_start(wt_sb[:], wt_in[:])
        nc.sync.dma_start(id_sb[:], id_in[:])
        nc.sync.dma_start(bias_sb[:], bias_in[:])
        kv_sb = wk.tile([1, L], FP32)
        wg_sb = wk.tile([1, 1], FP32)
        nc.sync.dma_start(kv_sb[:], kv_in[:])
        nc.sync.dma_start(wg_sb[:], wg_in[:])

        nc.vector.memset(ones1[:], 1.0)
        nc.vector.memset(ones_t[:], 1.0)
        nc.vector.memset(onesb[:], 1.0)
        nc.vector.memset(upd[:], 0.0)
        nc.vector.tensor_copy(cur[:], xs[:])

        # ---- degree -> dis = (max(deg,1))^-1/2 ----
        deg = wk.tile([P, NB], FP32)
        for b in range(NB):
            psd = ps_m.tile([P, 1], FP32, space="PSUM", tag="misc")
            for k in range(C):
                nc.tensor.matmul(
                    psd[:], s01_sb[:, (b * C + k) * P:(b * C + k + 1) * P],
                    onesb[:], start=(k == 0), stop=(k == C - 1))
            nc.vector.tensor_copy(deg[:, b:b + 1], psd[:])
        degm = wk.tile([P, NB], FP32)
        nc.vector.tensor_scalar_max(degm[:], deg[:], 1.0)
        rec = wk.tile([P, NB], FP32)
        nc.vector.reciprocal(rec[:], degm[:])
        nc.scalar.activation(dis[:], rec[:], mybir.ActivationFunctionType.Sqrt)
        nc.vector.tensor_scalar_mul(ndis[:], dis[:], -1.0)
        for b in range(NB):
            nc.vector.tensor_scalar_mul(
                disx[:, b * F:(b + 1) * F], ones_t[:], dis[:, b:b + 1])

        # ---- broadcast tanh(k) and sigmoid(weighting) to all partitions ----
        psb = ps_m.tile([P, L], FP32, space="PSUM", tag="misc")
        nc.tensor.matmul(psb[:], ones1[:], kv_sb[:], start=True, stop=True)
        nc.scalar.activation(tanhk[:], psb[:], mybir.ActivationFunctionType.Tanh)
        psb1 = ps_m.tile([P, 1], FP32, space="PSUM", tag="misc")
        nc.tensor.matmul(psb1[:], ones1[:], wg_sb[:], start=True, stop=True)
        nc.scalar.activation(cbc[:], psb1[:], mybir.ActivationFunctionType.Sigmoid)

        # ---- layers ----
        for i in range(nlayers):
            nc.vector.tensor_tensor(y_sb[:], cur[:], disx[:], mybir.AluOpType.mult)
            nc.sync.dma_start(
                y_in[:].rearrange("(b p) f -> p b f", p=P),
                y_sb[:].rearrange("p (b f) -> p b f", f=F))
            if SKIP_CC:
                nc.sync.dma_start(y_out[:NPC, :], y_in[:])
            else:
                nc.gpsimd.collective_compute(
                    "AllGather", mybir.AluOpType.bypass,
                    replica_groups=[list(range(NC))],
                    ins=[y_in[:].opt()], outs=[y_out[:].opt()])
            for b in range(NB):
                msg = msgp.tile([P, C * F], BF16, tag="msg")
                if SKIP_GATHER:
                    nc.vector.memset(msg[:], 0.0)
                else:
                    nc.gpsimd.dma_gather(
                        msg[:].rearrange("p (c f) -> p c f", f=F),
                        y_out[:],
                        gidx_sb[:, b * C * 8:(b + 1) * C * 8],
                        C * P, C * P, F, single_packet=False)
                ps = ps_agg.tile([P, F], FP32, space="PSUM")
                for k in range(C):
                    nc.tensor.matmul(
                        ps[:], s01_sb[:, (b * C + k) * P:(b * C + k + 1) * P],
                        msg[:, k * F:(k + 1) * F], start=(k == 0), stop=(k == C - 1))
                # cur_b = cur_b + ps * (-dis_b)
                nc.vector.scalar_tensor_tensor(
                    cur[:, b * F:(b + 1) * F], ps[:], ndis[:, b:b + 1],
                    cur[:, b * F:(b + 1) * F],
                    op0=mybir.AluOpType.mult, op1=mybir.AluOpType.add)
            # upd += tanh(k_i) * cur
            nc.vector.scalar_tensor_tensor(
                upd[:], cur[:], tanhk[:, i:i + 1], upd[:],
                op0=mybir.AluOpType.mult, op1=mybir.AluOpType.add)

        # ---- epilogue: h = x + c*(upd - x); outT = relu(W @ hT + b) ----
        t1 = per.tile([P, NB * F], FP32)
        nc.vector.tensor_tensor(t1[:], upd[:], xs[:], mybir.AluOpType.subtract)
        h = per.tile([P, NB * F], FP32)
        nc.vector.scalar_tensor_tensor(
            h[:], t1[:], cbc[:, 0:1], xs[:],
            op0=mybir.AluOpType.mult, op1=mybir.AluOpType.add)
        for b in range(NB):
            pst = ps_f.tile([P, 512], FP32, space="PSUM", tag="fin")
            nc.tensor.transpose(pst[:, :P], h[:, b * F:(b + 1) * F], id_sb[:])
            nc.vector.tensor_copy(hT[:, b * F:(b + 1) * F], pst[:, :P])
        nslices = -(-NPC // 512)
        for s in range(nslices):
            ns = min(512, NPC - s * 512)
            psf = ps_f.tile([P, 512], FP32, space="PSUM", tag="fin")
            nc.tensor.matmul(psf[:, :ns], wt_sb[:], hT[:, s * 512:s * 512 + ns],
                             start=True, stop=True)
            nc.scalar.activation(outsb[:, s * 512:s * 512 + ns], psf[:, :ns],
                                 mybir.ActivationFunctionType.Relu,
                                 bias=bias_sb[:, 0:1])
        nc.sync.dma_start(out_t[:], outsb[:])

    nc.compile()
    return nc


def kernel(x, edge_index, k_values, weighting, W, b):
    global LAST_EXEC_TIME_NS
    import os
    x = np.asarray(x)
    per_core, C, NB, NPC, NTOT, N = preprocess(x, edge_index)
    nc = build_program(C, NB)

    wt = np.ascontiguousarray(np.asarray(W, dtype=np.float32).T)
    bias = np.asarray(b, dtype=np.float32).reshape(F, 1)
    kv = np.asarray(k_values, dtype=np.float32).reshape(1, L)
    wg = np.asarray(weighting, dtype=np.float32).reshape(1, 1)
    ident = np.eye(P, dtype=np.float32)

    in_maps = [
        {"x_sl": pc["x_sl"], "s01": pc["s01"], "gidx": pc["gidx"],
         "wt": wt, "bias": bias, "kv": kv, "wg": wg, "ident": ident}
        for pc in per_core
    ]

    trace = bool(os.environ.get("BASS_TRACE"))
    if trace:
        _install_ntff_hook()
    res = run_bass_kernel_spmd(nc, in_maps, core_ids=list(range(NC)))
    LAST_EXEC_TIME_NS = res.exec_time_ns

    out = np.empty((N, F), dtype=np.float32)
    for r in range(NC):
        lo = r * NPC
        hi = min(N, lo + NPC)
        if hi > lo:
            out[lo:hi] = res.results[r]["outT"].T[: hi - lo]
    return out


# revision 10
# speedup vs baseline: 4.6704x; 4.6704x over previous
"""Trainium2 Bass kernel for nn_PSN (gnn_message_passing), 8 NeuronCores.

Math (per reference):
    deg  = segment_sum(ones, col); deg[deg==0] = 1; dis = deg^-1/2
    repeat L times:  agg = scatter_add(col, dis[row]*dis[col]*cur[row]);
                     cur = cur - agg; update += tanh(k_i)*cur
    c = sigmoid(weighting); h = c*update + (1-c)*x; out = relu(h @ W.T + b)

Strategy (v2, block-dense SpMM): shard target nodes across 8 cores
(1280/core).  The adjacency is encoded host-side as a dense grid of
128x128 count blocks in fp8 (exact small ints) and streamed from HBM
each layer -- no per-edge DMA descriptors.  Per layer on device:
    y = dis*cur -> AllGather (bf16) -> y_sb [128 src, 80 rb, 128 f]
    for rb in 80:  lhsT = y_rb (stationary);  psum_g[feat, tgt] +=
        y_rb.T @ A[rb, :, g*512:(g+1)*512]   (3 moving matmuls, fp8 A)
    drain: curT += psum * (-disxT);  updT += tanh(k_i)*curT
State is kept feature-major (curT [128 f, 1280 n]) so the final linear
layer runs without transposes; per-node scales use a pre-expanded
disxT [f, n] tensor.
"""
import sys
import types
import numpy as np
import ml_dtypes
from contextlib import ExitStack

import concourse.bass as bass
import concourse.tile as tile
from concourse import bacc, mybir
from concourse.bass_utils import run_bass_kernel_spmd

P = 128          # partitions / block size
NC = 8           # cores
F = 128          # feature dim (must equal P)
L = 8            # layers

FP32 = mybir.dt.float32
BF16 = mybir.dt.bfloat16
FP8 = mybir.dt.float8e4

LAST_EXEC_TIME_NS = None


def _install_ntff_hook():
    try:
        if "antenv.axon_hooks" in sys.modules:
            return
        import antenv
        from trn_agent_boot.trn_boot import _ntff_profile_via_ctypes

        m = types.ModuleType("antenv.axon_hooks")
        _state = {"hook": _ntff_profile_via_ctypes("/opt/axon/libaxon_pjrt.so")}
        m.set_axon_ntff_profile_hook = lambda h: _state.__setitem__("hook", h)
        m.get_axon_ntff_profile_hook = lambda: _state["hook"]
        sys.modules["antenv.axon_hooks"] = m
        antenv.axon_hooks = m
    except Exception:
        pass


def preprocess(x, edge_index):
    """Host-side index/layout prep: block-count adjacency (fp8), degree
    histogram, and feature-major x slices."""
    N = x.shape[0]
    NB = -(-N // (NC * P))           # target blocks per core
    NPC = NB * P
    NTOT = NC * NPC
    RB = NTOT // P                   # source blocks (global)

    row = np.asarray(edge_index[0], dtype=np.int64)
    col = np.asarray(edge_index[1], dtype=np.int64)

    A2 = np.zeros((NTOT, NTOT), dtype=np.int16)
    np.add.at(A2, (row, col), 1)

    deg = np.zeros(NTOT, dtype=np.float32)
    deg[:N] = np.bincount(col, minlength=N).astype(np.float32)

    x_pad = np.zeros((NTOT, F), dtype=np.float32)
    x_pad[:N] = np.asarray(x, dtype=np.float32)

    per_core = []
    for r in range(NC):
        cs = slice(r * NPC, (r + 1) * NPC)
        a_r = np.ascontiguousarray(
            A2[:, cs].reshape(RB, P, NPC)).astype(ml_dtypes.float8_e4m3)
        xsT = np.ascontiguousarray(x_pad[cs].T)            # [F, NPC]
        deg_nm = np.ascontiguousarray(
            deg[cs].reshape(NB, P).T)                      # [128, NB]
        deg_row = np.ascontiguousarray(deg[cs].reshape(1, NPC))
        per_core.append({"a": a_r, "xsT": xsT, "deg_nm": deg_nm,
                         "deg_row": deg_row})
    return per_core, NB, NPC, NTOT, N


def build_program(NB):
    NPC = NB * P
    NTOT = NC * NPC
    RB = NTOT // P
    RB_CHUNK = 8                      # source blocks per A-stream DMA
    assert RB % RB_CHUNK == 0
    ngr = -(-NPC // 512)              # psum groups over target nodes
    gsz = [min(512, NPC - g * 512) for g in range(ngr)]

    nc = bacc.Bacc("TRN2", target_bir_lowering=False, debug=False,
                   enable_asserts=False, num_devices=NC)

    a_in = nc.dram_tensor("a", [RB, P, NPC], FP8, kind="ExternalInput")
    x_in = nc.dram_tensor("xsT", [F, NPC], FP32, kind="ExternalInput")
    dnm_in = nc.dram_tensor("deg_nm", [P, NB], FP32, kind="ExternalInput")
    drow_in = nc.dram_tensor("deg_row", [1, NPC], FP32, kind="ExternalInput")
    wt_in = nc.dram_tensor("wt", [F, F], FP32, kind="ExternalInput")      # W.T
    bias_in = nc.dram_tensor("bias", [F, 1], FP32, kind="ExternalInput")
    kv_in = nc.dram_tensor("kv", [1, L], FP32, kind="ExternalInput")
    wg_in = nc.dram_tensor("wg", [1, 1], FP32, kind="ExternalInput")
    id_in = nc.dram_tensor("ident", [P, P], FP32, kind="ExternalInput")
    out_t = nc.dram_tensor("outT", [F, NPC], FP32, kind="ExternalOutput")

    with tile.TileContext(nc) as tc, ExitStack() as ctx:
        per = ctx.enter_context(tc.tile_pool(name="per", bufs=1))
        ap_pool = ctx.enter_context(tc.tile_pool(name="ap", bufs=3))
        wk = ctx.enter_context(tc.tile_pool(name="wk", bufs=2))
        ps_agg = ctx.enter_context(tc.tile_pool(name="ps_agg", bufs=1, space="PSUM"))
        ps_m = ctx.enter_context(tc.tile_pool(name="ps_m", bufs=2, space="PSUM"))
        ps_f = ctx.enter_context(tc.tile_pool(name="ps_f", bufs=2, space="PSUM"))
        dram = ctx.enter_context(tc.tile_pool(name="dram", bufs=1, space="DRAM"))

        # persistent state (feature-major)
        xsT = per.tile([F, NPC], FP32)
        curT = per.tile([F, NPC], FP32)
        updT = per.tile([F, NPC], FP32)
        disxT = per.tile([F, NPC], FP32)     # disxT[f, n] = dis[n]
        ndisxT = per.tile([F, NPC], FP32)
        dis_nm = per.tile([P, NB], FP32)     # node-major dis for y production
        y_sb = per.tile([P, RB * F], BF16)   # gathered y, [src p, rb, f]
        yn = per.tile([P, NB * F], BF16)     # own y slice, node-major
        tanhk = per.tile([P, L], FP32)
        cbc = per.tile([P, 1], FP32)
        wt_sb = per.tile([F, F], FP32)
        id_sb = per.tile([P, P], FP32)
        bias_sb = per.tile([F, 1], FP32)
        ones1 = per.tile([1, P], FP32)
        outsb = per.tile([F, NPC], FP32)

        y_in = dram.tile([NPC, F], BF16)
        y_out_t = nc.dram_tensor("y_out_sh", [NTOT, F], BF16, addr_space="Shared")
        y_out = y_out_t.ap()

        # ---- loads ----
        nc.sync.dma_start(xsT[:], x_in[:])
        nc.sync.dma_start(dis_nm[:], dnm_in[:])   # holds deg for now
        nc.sync.dma_start(wt_sb[:], wt_in[:])
        nc.sync.dma_start(id_sb[:], id_in[:])
        nc.sync.dma_start(bias_sb[:], bias_in[:])
        kv_sb = wk.tile([1, L], FP32)
        wg_sb = wk.tile([1, 1], FP32)
        drow = wk.tile([1, NPC], FP32)
        nc.sync.dma_start(kv_sb[:], kv_in[:])
        nc.sync.dma_start(wg_sb[:], wg_in[:])
        nc.sync.dma_start(drow[:], drow_in[:])

        nc.vector.memset(ones1[:], 1.0)
        nc.vector.memset(updT[:], 0.0)
        nc.vector.tensor_copy(curT[:], xsT[:])

        # ---- dis = (max(deg,1))^-1/2, both layouts ----
        nc.vector.tensor_scalar_max(dis_nm[:], dis_nm[:], 1.0)
        nc.vector.reciprocal(dis_nm[:], dis_nm[:])
        nc.scalar.activation(dis_nm[:], dis_nm[:], mybir.ActivationFunctionType.Sqrt)

        nc.vector.tensor_scalar_max(drow[:], drow[:], 1.0)
        nc.vector.reciprocal(drow[:], drow[:])
        nc.scalar.activation(drow[:], drow[:], mybir.ActivationFunctionType.Sqrt)
        # broadcast dis row across partitions: psum = ones1.T @ drow
        for g in range(ngr):
            psd = ps_m.tile([P, 512], FP32, space="PSUM", tag="misc")
            nc.tensor.matmul(psd[:, :gsz[g]], ones1[:],
                             drow[:, g * 512:g * 512 + gsz[g]],
                             start=True, stop=True)
            nc.vector.tensor_copy(disxT[:, g * 512:g * 512 + gsz[g]],
                                  psd[:, :gsz[g]])
        nc.vector.tensor_scalar_mul(ndisxT[:], disxT[:], -1.0)

        # ---- broadcast tanh(k) and sigmoid(weighting) ----
        psb = ps_m.tile([P, 512], FP32, space="PSUM", tag="misc")
        nc.tensor.matmul(psb[:, :L], ones1[:], kv_sb[:], start=True, stop=True)
        nc.scalar.activation(tanhk[:], psb[:, :L], mybir.ActivationFunctionType.Tanh)
        psb1 = ps_m.tile([P, 512], FP32, space="PSUM", tag="misc")
        nc.tensor.matmul(psb1[:, :1], ones1[:], wg_sb[:], start=True, stop=True)
        nc.scalar.activation(cbc[:], psb1[:, :1], mybir.ActivationFunctionType.Sigmoid)

        # ---- layers ----
        for i in range(L):
            # y (own slice, node-major): transpose curT blocks, scale by dis
            for b in range(NB):
                pst = ps_f.tile([P, 512], FP32, space="PSUM", tag="fin")
                nc.tensor.transpose(pst[:, :P], curT[:, b * F:(b + 1) * F], id_sb[:])
                nc.vector.tensor_scalar_mul(
                    yn[:, b * F:(b + 1) * F], pst[:, :P], dis_nm[:, b:b + 1])
            nc.sync.dma_start(
                y_in[:].rearrange("(b p) f -> p b f", p=P),
                yn[:].rearrange("p (b f) -> p b f", f=F))
            nc.gpsimd.collective_compute(
                "AllGather", mybir.AluOpType.bypass,
                replica_groups=[list(range(NC))],
                ins=[y_in[:].opt()], outs=[y_out[:].opt()])
            nc.sync.dma_start(
                y_sb[:].rearrange("p (rb f) -> p rb f", f=F),
                y_out[:].rearrange("(rb p) f -> p rb f", p=P))

            psg = []
            for _g in range(ngr):
                psgt = ps_agg.tile([P, 512], FP32, space="PSUM", tag=f"agg{_g}")
                psg.append(psgt)
            for rc in range(RB // RB_CHUNK):
                at = ap_pool.tile([P, RB_CHUNK * NPC], FP8, tag="at")
                nc.sync.dma_start(
                    at[:].rearrange("p (rb n) -> p rb n", n=NPC),
                    a_in[rc * RB_CHUNK:(rc + 1) * RB_CHUNK, :, :]
                    .rearrange("rb p n -> p rb n"))
                for rl in range(RB_CHUNK):
                    rb = rc * RB_CHUNK + rl
                    for g in range(ngr):
                        nc.tensor.matmul(
                            psg[g][:, :gsz[g]],
                            y_sb[:, rb * F:(rb + 1) * F],
                            at[:, rl * NPC + g * 512: rl * NPC + g * 512 + gsz[g]],
                            start=(rb == 0), stop=(rb == RB - 1))
            for g in range(ngr):
                sl = slice(g * 512, g * 512 + gsz[g])
                # curT += psum * (-disxT)
                tmp = wk.tile([P, 512], FP32, tag="tmp")
                nc.vector.tensor_tensor(tmp[:, :gsz[g]], psg[g][:, :gsz[g]],
                                        ndisxT[:, sl], mybir.AluOpType.mult)
                nc.vector.tensor_tensor(curT[:, sl], curT[:, sl],
                                        tmp[:, :gsz[g]], mybir.AluOpType.add)
            nc.vector.scalar_tensor_tensor(
                updT[:], curT[:], tanhk[:, i:i + 1], updT[:],
                op0=mybir.AluOpType.mult, op1=mybir.AluOpType.add)

        # ---- epilogue ----
        t1 = per.tile([F, NPC], FP32)
        nc.vector.tensor_tensor(t1[:], updT[:], xsT[:], mybir.AluOpType.subtract)
        h = per.tile([F, NPC], FP32)
        nc.vector.scalar_tensor_tensor(
            h[:], t1[:], cbc[:, 0:1], xsT[:],
            op0=mybir.AluOpType.mult, op1=mybir.AluOpType.add)
        for g in range(ngr):
            psf = ps_f.tile([P, 512], FP32, space="PSUM", tag="fin")
            nc.tensor.matmul(psf[:, :gsz[g]], wt_sb[:],
                             h[:, g * 512:g * 512 + gsz[g]], start=True, stop=True)
            nc.scalar.activation(outsb[:, g * 512:g * 512 + gsz[g]], psf[:, :gsz[g]],
                                 mybir.ActivationFunctionType.Relu,
                                 bias=bias_sb[:, 0:1])
        nc.sync.dma_start(out_t[:], outsb[:])

    nc.compile()
    return nc


def kernel(x, edge_index, k_values, weighting, W, b):
    global LAST_EXEC_TIME_NS
    import os
    x = np.asarray(x)
    per_core, NB, NPC, NTOT, N = preprocess(x, edge_index)
    nc = build_program(NB)

    wt = np.ascontiguousarray(np.asarray(W, dtype=np.float32).T)
    bias = np.asarray(b, dtype=np.float32).reshape(F, 1)
    kv = np.asarray(k_values, dtype=np.float32).reshape(1, L)
    wg = np.asarray(weighting, dtype=np.float32).reshape(1, 1)
    ident = np.eye(P, dtype=np.float32)

    in_maps = [
        {"a": pc["a"], "xsT": pc["xsT"], "deg_nm": pc["deg_nm"],
         "deg_row": pc["deg_row"],
         "wt": wt, "bias": bias, "kv": kv, "wg": wg, "ident": ident}
        for pc in per_core
    ]

    if os.environ.get("BASS_TRACE"):
        _install_ntff_hook()
    res = run_bass_kernel_spmd(nc, in_maps, core_ids=list(range(NC)))
    LAST_EXEC_TIME_NS = res.exec_time_ns

    out = np.empty((N, F), dtype=np.float32)
    for r in range(NC):
        lo = r * NPC
        hi = min(N, lo + NPC)
        if hi > lo:
            out[lo:hi] = res.results[r]["outT"].T[: hi - lo]
    return out
